# revision 19
# baseline (speedup 1.0000x reference)
"""Trainium2 Bass kernel for nn_AbstractFullyConnected (DeepPoly abstract
interpretation of a 5-layer MLP, FC = [784, 2048, 2048, 2048, 2048, 10]).

Strategy (8 NeuronCores, tensor-parallel):
  * Each layer-i bound computation is a back-substitution chain of GEMMs.
    The chain state is kept TRANSPOSED ("G-form": G[k, r] = M[r, k]) so the
    natural [out, in] weight layout serves directly as the matmul stationary
    operand (out = lhsT.T @ rhs) and no per-step transposes are needed.
  * The chain's output rows (layer-i out dim, 2048) are sharded 256/core; the
    low and high chains are stacked along the free dim (2 x 256 = 512 cols),
    so every chain GEMM is [K=2048] x [M=2048 or 784] x [N=512] per core.
  * All matmul operands are bf16 (fp32 PSUM accumulation): same PE rate as
    float32r, half the HBM/SBUF traffic.  W1/W2 stay RESIDENT in SBUF across
    all chains (loaded once); W3 is streamed during chain 4 only.
  * GEMM loops run k-outer over j-groups of 4 PSUM banks, so each chain
    step's first matmul depends only on the first scaled k-tile of the state
    -> the ReLU-backsub scaling pipeline overlaps the GEMM instead of
    serializing in front of it.
  * Bias/x accumulation uses "row form": lhsT is a [128,1] coefficient
    vector (1-cycle weight load), rhs is a packed state/reluP tile, out is a
    [1, N] PSUM row accumulated across the whole chain.  Signs are baked into
    the packed tiles (D_signed = c1*min(G,0)) so lo and hi accumulate from a
    single matmul each.  This replaces ~1500 N=1 matmuls (each with a hidden
    128-column LDWEIGHTS) with ~250 vector-stationary matmuls.
  * Final (l=1) box steps use  lo += S@mid0 + T@half0  with
    T = [-|S_lo| | +|S_hi|]  (mid0/half0 = input box center/radius).
  * After each layer only the bound VECTORS (x/low/high) are AllGathered
    (~3 KB) - no large collectives anywhere.
  * Layer-5 chain (10 outputs) shards each GEMM's out dim across cores with a
    small AllGather per step; its weights are prefetched during chain 4.
"""

import contextlib
import os

import numpy as np

MEAN, STD = 0.1307, 0.3081
N_CORES = 8
R = 256          # chain rows per core (2048 / 8)
NT = 16          # k-tiles for 2048
NT1 = 7          # k-tiles for 784 (padded to 896)
IN1P = 896
GRP = 4          # j-tiles per PSUM group in the chain GEMMs

# fp8e4m3 scaling: weights are stored as W*SW, chain states as SG*M.  All
# backsub state matrices measure max|.| <= 0.2 on the reference data, and
# |W| <= 0.2, so 512x scaling puts everything in ~[3e-5, 102] - comfortably
# inside TRN fp8_e4m3's [0.0156(normal)/0.002(subnormal), 240] envelope.
SW = 512.0       # weight scale (W1/W2/W3 fp8 residents)
SG = 512.0       # state scale (git start states + Gs chain states)
INV_SW = 1.0 / SW
INV_SG = 1.0 / SG
INV_SWSG = 1.0 / (SW * SG)

_CACHE = {}


# ----------------------------------------------------------------------------
# walrus in this container supports only ONE sync-wait per instruction; hoist
# extra waits emitted by the Tile scheduler into standalone single-wait
# EventSemaphore instructions placed just before the owning instruction.
# ----------------------------------------------------------------------------
def _split_multiwaits(nc):
    import concourse.mybir as mybir

    n = 0
    for f in nc.m.functions:
        for b in f.blocks:
            insts = list(b.instructions)
            if not any(
                (i.sync_info is not None and len(i.sync_info.on_wait) > 1)
                for i in insts
            ):
                continue
            new = []
            for i in insts:
                si = i.sync_info
                if si is not None and len(si.on_wait) > 1:
                    waits = list(si.on_wait)
                    for k, w in enumerate(waits[:-1]):
                        ev = mybir.InstEventSemaphore(
                            name=f"{i.name}_hw{k}", ins=[], outs=[]
                        )
                        ev.engine = i.engine
                        ev.sync_info = mybir.SyncInfo(on_wait=[w], on_update=[])
                        new.append(ev)
                        n += 1
                    i.sync_info = mybir.SyncInfo(
                        on_wait=[waits[-1]], on_update=list(si.on_update)
                    )
                new.append(i)
            b.instructions = new
    return n


def build_nc():
    KREP = int(os.environ.get("BASS_KREP", "1"))
    NOCC = bool(int(os.environ.get("BASS_NOCC", "0")))
    import concourse.bass as bass
    import concourse.mybir as mybir
    import concourse.tile as tile

    F32 = mybir.dt.float32
    BF = mybir.dt.bfloat16
    F8 = mybir.dt.float8e4
    AF = mybir.ActivationFunctionType
    ALU = mybir.AluOpType
    PM = mybir.MatmulPerfMode

    nc = bass.Bass("TRN2", target_bir_lowering=False, debug=False,
                   num_devices=N_CORES)

    # ---------------- DRAM I/O ----------------
    def din(name, shape, dt=BF):
        return nc.dram_tensor(name, shape, dt, kind="ExternalInput").ap()

    # all weight tensors arrive HOST-PRE-TRANSPOSED into the exact SBUF
    # layout (partition-major, contiguous >=512B chunks -> full DMA bw)
    W1r = din("W1r", [128, NT1 * NT * 128], F8)
    W2r = din("W2r", [128, NT * NT * 128], F8)
    W3r = din("W3r", [NT, 128, 2048], F8)
    Wshr = {l: din(f"W{l}shr", [2, 128, 2048]) for l in (2, 3, 4)}
    W1sh5r = din("W1sh5r", [1, 128, 2048])
    GTr = {i: din(f"G{i}Tr", [128, NT * R]) for i in (2, 3, 4)}
    G1Tr = din("G1Tr", [128, NT1 * R])
    W5r = din("W5r", [128, 160])
    bc = {l: din(f"bc{l}", [128, 16]) for l in (1, 2, 3, 4)}
    # b_l / SG, for the bias-vs-b_l matmuls whose lhsT is the SG-scaled
    # fp8 chain state
    bcsc = {l: din(f"bcsc{l}", [128, 16]) for l in (1, 2, 3, 4)}
    btil = {i: din(f"btil{i}", [128, 16], F32) for i in (1, 2, 3, 4)}
    b5d = din("b5", [10, 1], F32)
    # packed input vectors (host-normalized, bf16):
    #   V3 cols 3t+{0,1,2} = (mid0, mid0, x0); V2 cols 2t+{0,1} = (-half0, half0)
    V3d = din("V3", [128, 3 * NT1])
    V2d = din("V2", [128, 2 * NT1])

    out_d = nc.dram_tensor("out", [3, 10], F32, kind="ExternalOutput").ap()

    # internal DRAM for collectives
    ag_in = {i: nc.dram_tensor(f"ag{i}_in", [2, 384], F32).ap()
             for i in (1, 2, 3, 4)}
    ag_out = {i: nc.dram_tensor(f"ag{i}_out", [16, 384], F32,
                                addr_space="Shared").ap()
              for i in (1, 2, 3, 4)}
    ag5_in = {l: nc.dram_tensor(f"ag5_{l}_in", [2, 2560], F32).ap()
              for l in (4, 3, 2)}
    ag5_out = {l: nc.dram_tensor(f"ag5_{l}_out", [16, 2560], F32,
                                 addr_space="Shared").ap()
               for l in (4, 3, 2)}
    ag5f_in = nc.dram_tensor("ag5f_in", [1, 2560], F32).ap()
    ag5f_out = nc.dram_tensor("ag5f_out", [8, 2560], F32,
                              addr_space="Shared").ap()
    rg = [list(range(N_CORES))]

    with tile.TileContext(nc) as tc, contextlib.ExitStack() as est:
        pool_c = est.enter_context(tc.tile_pool(name="const", bufs=1))
        pool_git = est.enter_context(tc.tile_pool(name="git", bufs=2))
        pool_w = est.enter_context(tc.tile_pool(name="wstream", bufs=3))
        pool_w5 = est.enter_context(tc.tile_pool(name="w5stream", bufs=7))
        pool_gs = est.enter_context(tc.tile_pool(name="gs", bufs=2))
        pool_ab = est.enter_context(tc.tile_pool(name="ab", bufs=2))
        pool_misc = est.enter_context(tc.tile_pool(name="misc", bufs=2))
        pool_ps = est.enter_context(tc.tile_pool(name="ps", bufs=6, space="PSUM"))
        pool_bb = est.enter_context(tc.tile_pool(name="bb", bufs=1, space="PSUM"))

        # ---------------- constants / resident weights ----------------
        V3 = pool_c.tile([128, 3 * NT1], BF, tag="V3", name="V3")
        V2 = pool_c.tile([128, 2 * NT1], BF, tag="V2", name="V2")
        nc.sync.dma_start(V3[:, :], V3d[:, :])
        nc.sync.dma_start(V2[:, :], V2d[:, :])
        g1t = pool_c.tile([128, NT1 * R], BF, tag="g1t", name="g1t")
        nc.sync.dma_start(g1t[:, :], G1Tr[:, :])

        def mid0(t):
            return V3[:, 3 * t:3 * t + 1]

        def x0v(t):
            return V3[:, 3 * t + 2:3 * t + 3]

        def nhalf0(t):
            return V2[:, 2 * t:2 * t + 1]

        def half0(t):
            return V2[:, 2 * t + 1:2 * t + 2]

        bcs = {}
        for l in (1, 2, 3, 4):
            t = pool_c.tile([128, 16], BF, tag=f"bc{l}", name=f"bc{l}")
            nc.sync.dma_start(t[:, :], bc[l][:, :])
            bcs[l] = t
        bcscs = {}
        for l in (1, 2, 3, 4):
            t = pool_c.tile([128, 16], BF, tag=f"bcsc{l}", name=f"bcsc{l}")
            nc.sync.dma_start(t[:, :], bcsc[l][:, :])
            bcscs[l] = t
        btile = {}
        for i in (1, 2, 3, 4):
            t = pool_c.tile([128, 16], F32, tag=f"btil{i}", name=f"btil{i}")
            nc.sync.dma_start(t[:, :], btil[i][:, :])
            btile[i] = t
        b5t = pool_c.tile([10, 1], F32, tag="b5t", name="b5t")
        nc.sync.dma_start(b5t[:, :], b5d[:, :])

        # w5buf's DMA is emitted in the chain-4 prologue
        w5buf = pool_c.tile([128, 16 * 10], BF, tag="w5buf", name="w5buf")

        # chain-2 start state: prefetch before chain 1
        git = {}
        git[2] = pool_git.tile([128, NT * R], BF, tag="git", name="git2")
        for hh in range(2):
            nc.gpsimd.dma_start(git[2][:, hh * 2048:(hh + 1) * 2048],
                                GTr[2][:, hh * 2048:(hh + 1) * 2048])

        # resident full weights W1 (padded 896x2048) and W2 (2048x2048), bf16,
        # streamed on the Activation HWDGE queue so the latency-critical
        # gather/collective DMAs on the SP queue are never stuck behind them.
        W1res = pool_c.tile([128, NT1 * NT * 128], F8, tag="W1res", name="W1res")
        for j in range(NT1):
            nc.gpsimd.dma_start(W1res[:, j * 2048:(j + 1) * 2048],
                                W1r[:, j * 2048:(j + 1) * 2048])
        # W2 tile loads are spread through chain 2 (first needed in chain 3)
        W2res = pool_c.tile([128, NT * NT * 128], F8, tag="W2res", name="W2res")

        def load_w2res_tiles(js):
            for j in js:
                nc.gpsimd.dma_start(W2res[:, j * 2048:(j + 1) * 2048],
                                    W2r[:, j * 2048:(j + 1) * 2048])

        # per-layer relu coefficient tiles (filled after each layer)
        coef = {}
        for i in (1, 2, 3, 4):
            coef[i] = {}
            for k in ("c1", "c2", "nc2", "c1b", "c2b", "nc2b"):
                coef[i][k] = pool_c.tile([128, 16], F32, tag=f"cf{i}{k}",
                                         name=f"cf{i}{k}")
            for k in ("rhv", "xr", "rhv_s", "xr_s"):
                coef[i][k] = pool_c.tile([128, 16], BF, tag=f"cf{i}{k}",
                                         name=f"cf{i}{k}")

        # ---------------- helpers ----------------
        class BiasCols:
            """bias accumulation columns in one PSUM bank.
            cols 0,1: low m0/m1 | 2,3: high | 4,5: x  (col = 2*kind + m)"""

            def __init__(self, np_part=128):
                self.t = pool_bb.tile([128, 8], F32, tag="bbm", name="bbm")
                # start=True zeroes the WHOLE PSUM bank on this HW, so emit
                # exactly one start for the bank; later first-touches rely on
                # the bank-wide has_written clear (first write = overwrite).
                self.bank_first = True
                self.np_part = np_part

            def mm(self, col, lhsT, rhs, stop=False):
                nc.tensor.matmul(
                    self.t[0:self.np_part, col:col + 1], lhsT, rhs,
                    start=self.bank_first, stop=stop,
                )
                self.bank_first = False

        def relu_pass(dst, src, scale):
            nc.scalar.activation(dst, src, AF.Relu, scale=scale)

        def do_allgather(in_ap, out_ap, rows_per_rank):
            if NOCC:
                # timing-only stub: copy own shard into its slot
                nc.sync.dma_start(out_ap[0:rows_per_rank], in_ap[0:rows_per_rank])
            else:
                nc.gpsimd.collective_compute(
                    "AllGather", ALU.bypass, replica_groups=rg,
                    ins=[in_ap], outs=[out_ap])

        def gather_layer(i, br):
            """DMA bias cols out, AllGather, read back + add b_i, coeffs."""
            sh = pool_misc.tile([128, 6], F32, tag="sh", name="sh", bufs=1)
            nc.vector.tensor_copy(sh[:, :], br.t[:, 0:6])
            # ag row m = [lo(128) | hi(128) | x(128)]; sh col = 2*kind + m
            shr = sh[:, :].rearrange("p (k m) -> p m k", k=3)
            for mi in range(2):
                nc.sync.dma_start(
                    ag_in[i][mi:mi + 1, :].rearrange("a (k p) -> p (a k)",
                                                     k=3),
                    shr[:, mi, :],
                )
            do_allgather(ag_in[i][:, :], ag_out[i][:, :], 2)
            # X cols: [lo(16) | hi(16) | x(16)], t = ag row
            X = pool_misc.tile([128, 48], F32, tag="Xg", name="Xg")
            for c in range(3):
                nc.sync.dma_start(
                    X[:, 16 * c:16 * (c + 1)],
                    ag_out[i][:, 128 * c:128 * (c + 1)].rearrange("t p -> p t"),
                )
            for c in range(3):
                nc.vector.tensor_tensor(X[:, 16 * c:16 * (c + 1)],
                                        X[:, 16 * c:16 * (c + 1)],
                                        btile[i][:, :], ALU.add)
            compute_coeffs(i, X[:, 32:48], X[:, 0:16], X[:, 16:32])

        _cw = [0]

        def compute_coeffs(i, x, lo, hi):
            C = coef[i]

            def tmp():
                _cw[0] += 1
                return pool_misc.tile([128, 16], F32, tag=f"cw{_cw[0] % 20}",
                                      name=f"cw{_cw[0] % 20}", bufs=1)

            thp, tln, m = tmp(), tmp(), tmp()
            nc.vector.tensor_scalar(thp[:, :], hi[:, :], 0.0, None, ALU.is_gt)
            nc.vector.tensor_scalar(tln[:, :], lo[:, :], 0.0, None, ALU.is_lt)
            nc.vector.tensor_tensor(m[:, :], tln[:, :], thp[:, :], ALU.mult)
            onem = tmp()
            nc.vector.tensor_scalar(onem[:, :], m[:, :], -1.0, 1.0,
                                    ALU.mult, ALU.add)
            d, dm = tmp(), tmp()
            nc.vector.tensor_tensor(d[:, :], hi[:, :], lo[:, :], ALU.subtract)
            nc.vector.tensor_tensor(dm[:, :], d[:, :], m[:, :], ALU.mult)
            nc.vector.tensor_tensor(dm[:, :], dm[:, :], onem[:, :], ALU.add)
            r = tmp()
            nc.vector.reciprocal(r[:, :], dm[:, :])
            hr, k1 = tmp(), tmp()
            nc.vector.tensor_tensor(hr[:, :], hi[:, :], r[:, :], ALU.mult)
            nc.vector.tensor_tensor(hr[:, :], hr[:, :], m[:, :], ALU.mult)
            nc.vector.tensor_tensor(k1[:, :], thp[:, :], onem[:, :], ALU.mult)
            nc.vector.tensor_tensor(C["c1"][:, :], hr[:, :], k1[:, :], ALU.add)
            s, lam = tmp(), tmp()
            nc.vector.tensor_tensor(s[:, :], hi[:, :], lo[:, :], ALU.add)
            nc.vector.tensor_scalar(lam[:, :], s[:, :], 0.0, None, ALU.is_ge)
            nc.vector.tensor_tensor(lam[:, :], lam[:, :], m[:, :], ALU.mult)
            nc.vector.tensor_tensor(C["c2"][:, :], lam[:, :], k1[:, :], ALU.add)
            nc.vector.tensor_scalar_mul(C["nc2"][:, :], C["c2"][:, :], -1.0)
            # B-set: drain coefficients folding out the fp8 weight scale
            nc.vector.tensor_scalar_mul(C["c1b"][:, :], C["c1"][:, :], INV_SW)
            nc.vector.tensor_scalar_mul(C["c2b"][:, :], C["c2"][:, :], INV_SW)
            nc.vector.tensor_scalar_mul(C["nc2b"][:, :], C["c2b"][:, :], -1.0)
            # rhv = relu(-lo) * (hi > 0)   [== ub_int / ub_slope, no division]
            rl = tmp()
            nc.scalar.activation(rl[:, :], lo[:, :], AF.Relu, scale=-1.0)
            nc.vector.tensor_tensor(C["rhv"][:, :], rl[:, :], thp[:, :],
                                    ALU.mult)
            # rhv/SG + relu(x)/SG: partners of the SG-scaled AD / git tiles
            nc.vector.tensor_scalar_mul(C["rhv_s"][:, :], C["rhv"][:, :],
                                        INV_SG)
            nc.scalar.activation(C["xr"][:, :], x[:, :], AF.Relu)
            nc.scalar.activation(C["xr_s"][:, :], x[:, :], AF.Relu,
                                 scale=INV_SG)

        # pending small PE matmuls, drained a couple per k-iteration of the
        # subsequent GEMM loops so they interleave with the big-MM stream
        pending = []

        def drain_pending(nmax):
            npop = min(nmax, len(pending))
            for f in pending[:npop]:
                f()
            del pending[:npop]

        for _rep in range(KREP):
            # ================= chain 1 (layer 1) =================
            bb1 = BiasCols()
            for t in range(NT1):
                gsl = g1t[:, t * R:(t + 1) * R]
                T1 = pool_ab.tile([128, R], BF, tag="T1", name="T1", bufs=2)
                nc.scalar.activation(T1[:, :], gsl, AF.Abs)
                last = t == NT1 - 1
                for mi in range(2):
                    gm = g1t[:, t * R + mi * 128:t * R + (mi + 1) * 128]
                    tm = T1[:, mi * 128:(mi + 1) * 128]
                    bb1.mm(4 + mi, gm, x0v(t), stop=last)
                    bb1.mm(0 + mi, gm, mid0(t))
                    bb1.mm(0 + mi, tm, nhalf0(t), stop=last)
                    bb1.mm(2 + mi, gm, mid0(t))
                    bb1.mm(2 + mi, tm, half0(t), stop=last)
            gather_layer(1, bb1)

            # ================= chains 2..4 =================
            for i in (2, 3, 4):
                g = git[i]
                if i < 4:
                    # prefetch next chain's start state during this chain
                    git[i + 1] = pool_git.tile([128, NT * R], BF, tag="git",
                                               name=f"git{i + 1}")
                    for hh in range(2):
                        nc.gpsimd.dma_start(
                            git[i + 1][:, hh * 2048:(hh + 1) * 2048],
                            GTr[i + 1][:, hh * 2048:(hh + 1) * 2048])
                w3stream = []
                wb5 = {}
                if i == 4:
                    # stream W3 for the l=3 step (16 fp8 tiles, rolling bufs)
                    for j in range(NT):
                        wb = pool_w.tile([128, 2048], F8, tag="wb", name="wb",
                                         bufs=4)
                        nc.gpsimd.dma_start(wb[:, :], W3r[j])
                        w3stream.append(wb)
                    # prefetch chain-5 weight shards (bf16, own rolling pool)
                    for l in (4, 3, 2):
                        for jj in range(2):
                            w = pool_w5.tile([128, 2048], BF, tag="wb5",
                                             name=f"wb5_{l}_{jj}", bufs=7)
                            nc.gpsimd.dma_start(w[:, :], Wshr[l][jj])
                            wb5[(l, jj)] = w
                    w = pool_w5.tile([128, 2048], BF, tag="wb5", name="wb5_1",
                                     bufs=7)
                    nc.gpsimd.dma_start(w[:, :], W1sh5r[0])
                    wb5[(1, 0)] = w
                    nc.gpsimd.dma_start(w5buf[:, :], W5r[:, :])

                bbx = BiasCols()
                cf = coef[i - 1]

                # ---- first block: scale start state g by coef[i-1] ----
                # g arrives host-pre-scaled by SG, so AD/Gs inherit the SG
                # state scale; fb bias matmuls pair them with rhv_s/xr_s.
                # AD[t]: cols [0:256] = D_signed = c1*min(g,0) (<=0),
                #        cols [256:512] = A = c1*max(g,0)
                # Gs[t]: lo = c2*max(g,0) + D_signed ; hi = A - (-c2*min(g,0))
                Gs = pool_gs.tile([128, NT * 512], F8, tag="gs", name="gs")
                for t in range(NT):
                    gsl = g[:, t * R:(t + 1) * R]
                    AD = pool_ab.tile([128, 512], BF, tag="AD", name="AD",
                                      bufs=5)
                    B = pool_ab.tile([128, R], BF, tag="B", name="B", bufs=3)
                    Cc = pool_ab.tile([128, R], BF, tag="C", name="C", bufs=3)
                    relu_pass(AD[:, 256:512], gsl, cf["c1"][:, t:t + 1])
                    nc.vector.tensor_scalar(AD[:, 0:256], gsl, 0.0,
                                            cf["c1"][:, t:t + 1],
                                            ALU.min, ALU.mult)
                    relu_pass(Cc[:, :], gsl, cf["c2"][:, t:t + 1])
                    nc.vector.tensor_scalar(B[:, :], gsl, 0.0,
                                            cf["nc2"][:, t:t + 1],
                                            ALU.min, ALU.mult)
                    nc.vector.tensor_tensor(
                        Gs[:, t * 512:t * 512 + 256],
                        Cc[:, :], AD[:, 0:256], ALU.add)
                    nc.vector.tensor_tensor(
                        Gs[:, t * 512 + 256:t * 512 + 512],
                        AD[:, 256:512], B[:, :], ALU.subtract)

                    def fb_mms(t=t, AD=AD, gsl=gsl, cf=cf):
                        last = t == NT - 1
                        for mi in range(2):
                            sl = slice(mi * 128, (mi + 1) * 128)
                            sl2 = slice(256 + mi * 128, 256 + (mi + 1) * 128)
                            bbx.mm(0 + mi, AD[:, sl], cf["rhv_s"][:, t:t + 1])
                            bbx.mm(2 + mi, AD[:, sl2],
                                   cf["rhv_s"][:, t:t + 1])
                            bbx.mm(4 + mi, gsl[:, sl], cf["xr_s"][:, t:t + 1],
                                   stop=last)
                    pending.append(fb_mms)

                # ---- GEMM steps l = i-1 .. 1 ----
                for li, l in enumerate(range(i - 1, 0, -1)):
                    nj = NT if l > 1 else NT1
                    Gs_next = (pool_gs.tile([128, NT * 512], F8, tag="gs",
                                            name="gs") if l > 1 else None)
                    cfl = coef[l - 1] if l > 1 else None

                    def wsl2(j, kk, l=l):
                        # [128, 2, 128] fp8 stationary: two adjacent k-planes
                        if l == 1:
                            w = W1res[:, (j * NT + 2 * kk) * 128:
                                      (j * NT + 2 * kk + 2) * 128]
                        elif l == 2:
                            w = W2res[:, (j * NT + 2 * kk) * 128:
                                      (j * NT + 2 * kk + 2) * 128]
                        else:
                            w = w3stream[j][:, 2 * kk * 128:
                                            (2 * kk + 2) * 128]
                        return w.rearrange("p (two m) -> p two m", two=2)

                    # bias vs b_l on the current (scaled) state
                    for t in range(NT):
                        def bl_mms(t=t, Gs=Gs, l=l):
                            for mi in range(2):
                                lo_l = Gs[:, t * 512 + mi * 128:
                                          t * 512 + (mi + 1) * 128]
                                hi_l = Gs[:, t * 512 + 256 + mi * 128:
                                          t * 512 + 256 + (mi + 1) * 128]
                                bbx.mm(0 + mi, lo_l, bcscs[l][:, t:t + 1])
                                bbx.mm(2 + mi, hi_l, bcscs[l][:, t:t + 1])
                        pending.append(bl_mms)

                    groups = [list(range(g0, min(g0 + GRP, nj)))
                              for g0 in range(0, nj, GRP)]
                    for gi, grp in enumerate(groups):
                        if i == 2:
                            load_w2res_tiles(range(gi * 8, gi * 8 + 8))
                        pss = {}
                        for j in grp:
                            pss[j] = pool_ps.tile([128, 512], F32, tag="ps",
                                                  name="ps")
                        for kk in range(NT // 2):
                            for j in grp:
                                nc.tensor.matmul(
                                    pss[j][:, :], wsl2(j, kk),
                                    Gs[:, kk * 1024:(kk + 1) * 1024]
                                    .rearrange("p (two n) -> p two n", two=2),
                                    start=(kk == 0), stop=(kk == NT // 2 - 1),
                                    perf_mode=PM.DoubleRow)
                            drain_pending(4)
                        # drains + scaling for this group
                        for j in grp:
                            ps = pss[j]
                            if l > 1:
                                AD = pool_ab.tile([128, 512], BF, tag="AD",
                                                  name="AD", bufs=5)
                                B = pool_ab.tile([128, R], BF, tag="B",
                                                 name="B", bufs=3)
                                Cc = pool_ab.tile([128, R], BF, tag="C",
                                                  name="C", bufs=3)
                                hi_sl, lo_sl = ps[:, 256:512], ps[:, 0:256]
                                relu_pass(AD[:, 256:512], hi_sl,
                                          cfl["c1b"][:, j:j + 1])
                                nc.vector.tensor_scalar(AD[:, 0:256], lo_sl,
                                                        0.0,
                                                        cfl["c1b"][:, j:j + 1],
                                                        ALU.min, ALU.mult)
                                relu_pass(Cc[:, :], lo_sl,
                                          cfl["c2b"][:, j:j + 1])
                                nc.vector.tensor_scalar(B[:, :], hi_sl, 0.0,
                                                        cfl["nc2b"][:, j:j + 1],
                                                        ALU.min, ALU.mult)
                                nc.vector.tensor_tensor(
                                    Gs_next[:, j * 512:j * 512 + 256],
                                    Cc[:, :], AD[:, 0:256], ALU.add)
                                nc.vector.tensor_tensor(
                                    Gs_next[:, j * 512 + 256:j * 512 + 512],
                                    AD[:, 256:512], B[:, :], ALU.subtract)

                                def relu_mms(AD=AD, j=j, cfl=cfl):
                                    for mi in range(2):
                                        sl = slice(mi * 128, (mi + 1) * 128)
                                        sl2 = slice(256 + mi * 128,
                                                    256 + (mi + 1) * 128)
                                        bbx.mm(0 + mi, AD[:, sl],
                                               cfl["rhv_s"][:, j:j + 1])
                                        bbx.mm(2 + mi, AD[:, sl2],
                                               cfl["rhv_s"][:, j:j + 1])
                                pending.append(relu_mms)
                            else:
                                S = pool_ab.tile([128, 512], BF, tag="S",
                                                 name="S", bufs=3)
                                T = pool_ab.tile([128, 512], BF, tag="T",
                                                 name="T", bufs=3)
                                # PSUM carries SW*SG x true values here
                                nc.scalar.activation(S[:, :], ps[:, :],
                                                     AF.Copy, scale=INV_SWSG)
                                nc.scalar.activation(T[:, :], ps[:, :],
                                                     AF.Abs, scale=INV_SWSG)

                                def box_mms(S=S, T=T, j=j,
                                            last=(j == NT1 - 1)):
                                    for mi in range(2):
                                        sl = slice(mi * 128, (mi + 1) * 128)
                                        sl2 = slice(256 + mi * 128,
                                                    256 + (mi + 1) * 128)
                                        bbx.mm(0 + mi, S[:, sl], mid0(j))
                                        bbx.mm(0 + mi, T[:, sl], nhalf0(j),
                                               stop=last)
                                        bbx.mm(2 + mi, S[:, sl2], mid0(j))
                                        bbx.mm(2 + mi, T[:, sl2], half0(j),
                                               stop=last)
                                pending.append(box_mms)
                    if l > 1:
                        Gs = Gs_next
                drain_pending(len(pending))
                gather_layer(i, bbx)

            # ================= chain 5 (10-wide, sharded GEMM steps) ========
            cf4 = coef[4]
            G5s = pool_c.tile([128, 16 * 20], BF, tag="g5s", name="g5s")
            bb5 = BiasCols(np_part=10)
            for t in range(NT):
                gsl = w5buf[:, t * 10:(t + 1) * 10]
                AD5 = pool_ab.tile([128, 20], BF, tag="AD5", name="AD5",
                                   bufs=2)
                B5 = pool_ab.tile([128, 10], BF, tag="B5", name="B5", bufs=2)
                C5 = pool_ab.tile([128, 10], BF, tag="C5", name="C5", bufs=2)
                relu_pass(AD5[:, 10:20], gsl, cf4["c1"][:, t:t + 1])
                nc.vector.tensor_scalar(AD5[:, 0:10], gsl, 0.0,
                                        cf4["c1"][:, t:t + 1],
                                        ALU.min, ALU.mult)
                relu_pass(C5[:, :], gsl, cf4["c2"][:, t:t + 1])
                nc.vector.tensor_scalar(B5[:, :], gsl, 0.0,
                                        cf4["nc2"][:, t:t + 1],
                                        ALU.min, ALU.mult)
                nc.vector.tensor_tensor(G5s[:, t * 20:t * 20 + 10],
                                        C5[:, :], AD5[:, 0:10], ALU.add)
                nc.vector.tensor_tensor(G5s[:, t * 20 + 10:t * 20 + 20],
                                        AD5[:, 10:20], B5[:, :], ALU.subtract)
                bb5.mm(0, AD5[:, 0:10], cf4["rhv"][:, t:t + 1])
                bb5.mm(2, AD5[:, 10:20], cf4["rhv"][:, t:t + 1])
                bb5.mm(4, gsl, cf4["xr"][:, t:t + 1], stop=(t == NT - 1))

            for l in (4, 3, 2, 1):
                for t in range(NT):
                    bb5.mm(0, G5s[:, t * 20:t * 20 + 10], bcs[l][:, t:t + 1])
                    bb5.mm(2, G5s[:, t * 20 + 10:t * 20 + 20],
                           bcs[l][:, t:t + 1])
                if l > 1:
                    ps5 = pool_ps.tile([128, 512], F32, tag="ps",
                                       name="ps5")[:, 0:40]
                    for jj in range(2):
                        for k in range(NT):
                            nc.tensor.matmul(
                                ps5[:, jj * 20:(jj + 1) * 20],
                                wb5[(l, jj)][:, k * 128:(k + 1) * 128],
                                G5s[:, k * 20:(k + 1) * 20],
                                start=(jj == 0 and k == 0), stop=(k == NT - 1))
                    shc = pool_misc.tile([128, 40], F32, tag="shc5", name="shc5")
                    nc.scalar.copy(shc[:, :], ps5[:, :])
                    for jj in range(2):
                        nc.sync.dma_start(
                            ag5_in[l][jj:jj + 1, :].rearrange("a (p c) -> (a p) c", p=128),
                            shc[:, jj * 20:(jj + 1) * 20])
                    do_allgather(ag5_in[l][:, :], ag5_out[l][:, :], 2)
                    G5mm = pool_c.tile([128, 16 * 20], F32, tag="g5mm", name=f"g5mm{l}")
                    nc.sync.dma_start(
                        G5mm[:, :].rearrange("p (t c) -> p t c", t=NT),
                        ag5_out[l].rearrange("t (p c) -> p t c", p=128),
                    )
                    cfl = coef[l - 1]
                    G5n = pool_c.tile([128, 16 * 20], BF, tag="g5n", name=f"g5n{l}")
                    for t in range(NT):
                        gsl = G5mm[:, t * 20:(t + 1) * 20]
                        hi_sl = gsl[:, 10:20]
                        lo_sl = gsl[:, 0:10]
                        AD5 = pool_ab.tile([128, 20], BF, tag="AD5",
                                           name="AD5", bufs=2)
                        B5 = pool_ab.tile([128, 10], BF, tag="B5", name="B5",
                                          bufs=2)
                        C5 = pool_ab.tile([128, 10], BF, tag="C5", name="C5",
                                          bufs=2)
                        relu_pass(AD5[:, 10:20], hi_sl, cfl["c1"][:, t:t + 1])
                        nc.vector.tensor_scalar(AD5[:, 0:10], lo_sl, 0.0,
                                                cfl["c1"][:, t:t + 1],
                                                ALU.min, ALU.mult)
                        relu_pass(C5[:, :], lo_sl, cfl["c2"][:, t:t + 1])
                        nc.vector.tensor_scalar(B5[:, :], hi_sl, 0.0,
                                                cfl["nc2"][:, t:t + 1],
                                                ALU.min, ALU.mult)
                        nc.vector.tensor_tensor(G5n[:, t * 20:t * 20 + 10],
                                                C5[:, :], AD5[:, 0:10],
                                                ALU.add)
                        nc.vector.tensor_tensor(G5n[:, t * 20 + 10:t * 20 + 20],
                                                AD5[:, 10:20], B5[:, :],
                                                ALU.subtract)
                        bb5.mm(0, AD5[:, 0:10], cfl["rhv"][:, t:t + 1])
                        bb5.mm(2, AD5[:, 10:20], cfl["rhv"][:, t:t + 1])
                    G5s = G5n
                else:
                    ps5 = pool_ps.tile([128, 512], F32, tag="ps",
                                       name="ps5f")[:, 0:20]
                    for k in range(NT):
                        nc.tensor.matmul(
                            ps5[:, :],
                            wb5[(1, 0)][:, k * 128:(k + 1) * 128],
                            G5s[:, k * 20:(k + 1) * 20],
                            start=(k == 0), stop=(k == NT - 1))
                    shc = pool_misc.tile([128, 20], F32, tag="shc5f", name="shc5f")
                    nc.scalar.copy(shc[:, :], ps5[:, :])
                    nc.sync.dma_start(
                        ag5f_in[0:1, :].rearrange("a (p c) -> (a p) c", p=128),
                        shc[:, :])
                    do_allgather(ag5f_in[:, :], ag5f_out[:, :], 1)
                    G5f = pool_c.tile([128, NT1 * 20], F32, tag="g5f", name="g5f")
                    nc.sync.dma_start(
                        G5f[:, :].rearrange("p (t c) -> p t c", t=NT1),
                        ag5f_out[0:NT1].rearrange("t (p c) -> p t c", p=128),
                    )
                    for t in range(NT1):
                        gsl = G5f[:, t * 20:(t + 1) * 20]
                        S5 = pool_ab.tile([128, 20], BF, tag="AD5", name="AD5",
                                          bufs=2)
                        T5 = pool_ab.tile([128, 20], BF, tag="T5", name="T5",
                                          bufs=2)
                        nc.scalar.copy(S5[:, :], gsl)
                        nc.scalar.activation(T5[:, :], gsl, AF.Abs)
                        last = t == NT1 - 1
                        bb5.mm(0, S5[:, 0:10], mid0(t))
                        bb5.mm(0, T5[:, 0:10], nhalf0(t), stop=last)
                        bb5.mm(2, S5[:, 10:20], mid0(t))
                        bb5.mm(2, T5[:, 10:20], half0(t), stop=last)

            # final outputs: out[0]=x5, out[1]=low5, out[2]=high5
            fin = pool_misc.tile([10, 3], F32, tag="fin", name="fin")
            nc.vector.tensor_tensor(fin[:, 0:1], bb5.t[0:10, 4:5], b5t[:, :],
                                    ALU.add)
            nc.vector.tensor_tensor(fin[:, 1:2], bb5.t[0:10, 0:1], b5t[:, :],
                                    ALU.add)
            nc.vector.tensor_tensor(fin[:, 2:3], bb5.t[0:10, 2:3], b5t[:, :],
                                    ALU.add)
            nc.sync.dma_start(out_d.rearrange("k p -> p k"), fin[:, :])

    _split_multiwaits(nc)
    return nc


def make_in_maps(x, low, high, Ws, bs):
    """Host-side shard/layout prep. Ws/bs: dicts 1..5."""
    import ml_dtypes

    BFnp = ml_dtypes.bfloat16
    F8np = ml_dtypes.float8_e4m3

    def to_f8(a):
        s = np.asarray(a, np.float32) * SW
        assert np.abs(s).max() < 235.0, (
            f"fp8 weight scale overflow: max {np.abs(s).max()}")
        return s.astype(F8np)

    def pad_vec(v):
        p = np.zeros(IN1P, np.float32)
        p[:784] = ((np.asarray(v).reshape(-1) - MEAN) / STD)
        return p

    xn, ln, hn = pad_vec(x), pad_vec(low), pad_vec(high)
    mid = 0.5 * (ln + hn)
    half = 0.5 * (hn - ln)
    V3 = np.zeros((128, 3 * NT1), np.float32)
    V2 = np.zeros((128, 2 * NT1), np.float32)
    for t in range(NT1):
        seg = slice(t * 128, (t + 1) * 128)
        V3[:, 3 * t + 0] = mid[seg]
        V3[:, 3 * t + 1] = mid[seg]
        V3[:, 3 * t + 2] = xn[seg]
        V2[:, 2 * t + 0] = -half[seg]
        V2[:, 2 * t + 1] = half[seg]

    W1p = np.zeros((2048, IN1P), np.float32)
    W1p[:, :784] = Ws[1]

    def wres(W):
        # Wres[p, (j k c)] = W[k*128+p, j*128+c]
        nj = W.shape[1] // 128
        return np.ascontiguousarray(
            W.reshape(16, 128, nj, 128).transpose(1, 2, 0, 3)
            .reshape(128, nj * 16 * 128)).astype(BFnp)

    def wstream(W):
        # Wst[j, p, (k c)] = W[k*128+p, j*128+c]
        nj = W.shape[1] // 128
        return np.ascontiguousarray(
            W.reshape(16, 128, nj, 128).transpose(2, 1, 0, 3)
            .reshape(nj, 128, 16 * 128)).astype(BFnp)

    def gform(G):
        # g[p, (t c)] = G[t*128+p, c]   (G = [K, r])
        nt = G.shape[0] // 128
        return np.ascontiguousarray(
            G.reshape(nt, 128, G.shape[1]).transpose(1, 0, 2)
            .reshape(128, -1)).astype(BFnp)

    common = {"V3": V3.astype(BFnp), "V2": V2.astype(BFnp),
              "W5r": gform(np.ascontiguousarray(Ws[5].T)),
              "b5": np.ascontiguousarray(bs[5].reshape(10, 1)),
              "W1r": to_f8(wres(W1p).astype(np.float32)),
              "W2r": to_f8(wres(Ws[2]).astype(np.float32)),
              "W3r": to_f8(wstream(Ws[3]).astype(np.float32))}
    W2st = wstream(Ws[2])
    W4st = wstream(Ws[4])
    W1st = wstream(W1p)
    W3st = wstream(Ws[3])
    for l in (1, 2, 3, 4):
        common[f"bc{l}"] = np.ascontiguousarray(
            bs[l].reshape(16, 128).T).astype(BFnp)
        common[f"bcsc{l}"] = np.ascontiguousarray(
            bs[l].reshape(16, 128).T / SG).astype(BFnp)
        common[f"btil{l}"] = np.ascontiguousarray(
            bs[l].reshape(16, 128).T.astype(np.float32))

    maps = []
    for d in range(N_CORES):
        m = dict(common)
        sh = slice(256 * d, 256 * (d + 1))
        for i, W in ((2, Ws[2]), (3, Ws[3]), (4, Ws[4])):
            m[f"G{i}Tr"] = gform(
                np.ascontiguousarray(W[sh, :].T) * SG)
        m["G1Tr"] = gform(np.ascontiguousarray(W1p[sh, :].T))
        m["W2shr"] = np.ascontiguousarray(W2st[2 * d:2 * d + 2])
        m["W3shr"] = np.ascontiguousarray(W3st[2 * d:2 * d + 2])
        m["W4shr"] = np.ascontiguousarray(W4st[2 * d:2 * d + 2])
        m["W1sh5r"] = np.ascontiguousarray(W1st[min(d, 6):min(d, 6) + 1])
        maps.append(m)
    return maps


def _prep_inputs(inputs):
    Ws = {i: np.asarray(inputs[f"W{i}"], np.float32) for i in range(1, 6)}
    bs = {i: np.asarray(inputs[f"b{i}"], np.float32) for i in range(1, 6)}
    return make_in_maps(
        np.asarray(inputs["x"], np.float32),
        np.asarray(inputs["low"], np.float32),
        np.asarray(inputs["high"], np.float32),
        Ws, bs,
    )


def kernel(**inputs):
    from concourse import bass_utils

    if "nc" not in _CACHE:
        _CACHE["nc"] = build_nc()
    nc = _CACHE["nc"]

    in_maps = _prep_inputs(inputs)
    res = bass_utils.run_bass_kernel_spmd(nc, in_maps,
                                          core_ids=list(range(N_CORES)))
    out = res.results[0]["out"]
    return out[0].copy(), out[1].copy(), out[2].copy()


if __name__ == "__main__":
    import reference

    inp = reference.setup_inputs()
    inp_np = {k: np.asarray(v) for k, v in inp.items()}
    got = kernel(**inp_np)
    exp = reference.reference(**inp)
    for name, g, e in zip(("x", "low", "high"), got, exp):
        e = np.asarray(e)
        err = np.abs(g - e).max() / max(np.abs(e).max(), 1e-9)
        print(f"{name}: rel_err={err:.3e}")
        print("  got:", g[:5])
        print("  exp:", e[:5])

